# revision 1
# baseline (speedup 1.0000x reference)
"""Masked fractional Hamming distance over 31 circular rotations, on 8 trn2 cores.

Math: for shift s, num(s)/den(s) with
  den(s) = sum maskbits = corr(ma, mb)(2s)        (l,k fused -> lag 2s)
  num(s) = masked differing bits; with the sign-encode
  A = (ia<<7)|ma, B = (ib<<7)|mb read as fp8e4m3 the bytes become
  {+0, -0, +2^-9, -2^-9} (sign=iris, magnitude=mask), so
  corr(A, B)(2s) = (den - 2*num) * 2^-18, corr(ma, mb raw bytes) = den * 2^-18.
Both correlations are computed as banded matmuls on the PE: contraction over
rows (128/partition group), stationary = 128-column chunk of the A side,
moving = 188-column window of the (30-halo-padded) B side; every chunk and
row-group accumulates into one (128,188) PSUM tile per pair since the
diagonal offset d = j - i - 30 is tiling-invariant. Band diagonals are summed
on the host (exact integers scaled by 2^-18).
"""

import numpy as np

N_CORES = 8
B_FULL, L = 4096, 2048
R = 15
J = 2 * L                      # fused (l, k) axis, circular shifts = even lags
B_SH = B_FULL // N_CORES       # 512 batches per core
ROWS = 2 * B_SH                # 1024 rows of length J per core
HALO = 2 * R                   # 30
NW = 128 + 2 * HALO            # 188 moving window
N_GROUPS = ROWS // 128         # 8
N_CHUNKS = J // 128            # 32

_CACHE = {}


def _build_program():
    import concourse.bass as bass
    import concourse.tile as tile
    from concourse import bacc, mybir

    u8 = mybir.dt.uint8
    u16 = mybir.dt.uint16
    f8 = mybir.dt.float8e4
    f32 = mybir.dt.float32
    Alu = mybir.AluOpType

    nc = bass.Bass()
    ia_d = nc.declare_dram_parameter("ia", [ROWS, J], u8, isOutput=False)
    ma_d = nc.declare_dram_parameter("ma", [ROWS, J], u8, isOutput=False)
    ib_d = nc.declare_dram_parameter("ib", [ROWS, J], u8, isOutput=False)
    mb_d = nc.declare_dram_parameter("mb", [ROWS, J], u8, isOutput=False)
    out_d = nc.declare_dram_parameter("out", [2, 128, NW], f32, isOutput=True)

    with tile.TileContext(nc) as tc:
        with (
            tc.tile_pool(name="raw", bufs=3) as raw_pool,
            tc.tile_pool(name="enc", bufs=3) as enc_pool,
            tc.tile_pool(name="acc", bufs=1, space="PSUM") as psum_pool,
        ):
            ps_ab = psum_pool.tile([128, NW], f32)
            ps_mm = psum_pool.tile([128, NW], f32)

            for g in range(N_GROUPS):
                rows = slice(g * 128, (g + 1) * 128)
                ia_t = raw_pool.tile([128, J], u8, tag="ia")
                ma_t = raw_pool.tile([128, J], u8, tag="ma")
                ib_t = raw_pool.tile([128, J], u8, tag="ib")
                mb_t = raw_pool.tile([128, J + 2 * HALO], u8, tag="mb")
                a_t = enc_pool.tile([128, J], u8, tag="A")
                b_t = enc_pool.tile([128, J + 2 * HALO], u8, tag="B")

                nc.sync.dma_start(ia_t[:], ia_d[rows, :])
                nc.sync.dma_start(ma_t[:], ma_d[rows, :])
                nc.sync.dma_start(ib_t[:], ib_d[rows, :])
                nc.sync.dma_start(mb_t[:, HALO : HALO + J], mb_d[rows, :])

                # A = (ia << 7) | ma per byte, done on u16-viewed data (both
                # bytes of a pair are {0,1}: the shift never crosses bytes).
                nc.vector.tensor_scalar_mul(
                    a_t[:].bitcast(u16), ia_t[:].bitcast(u16), 128.0
                )
                nc.vector.tensor_tensor(
                    a_t[:].bitcast(u16),
                    a_t[:].bitcast(u16),
                    ma_t[:].bitcast(u16),
                    op=Alu.bitwise_or,
                )
                nc.vector.tensor_scalar_mul(
                    b_t[:, HALO : HALO + J].bitcast(u16),
                    ib_t[:].bitcast(u16),
                    128.0,
                )
                nc.vector.tensor_tensor(
                    b_t[:, HALO : HALO + J].bitcast(u16),
                    b_t[:, HALO : HALO + J].bitcast(u16),
                    mb_t[:, HALO : HALO + J].bitcast(u16),
                    op=Alu.bitwise_or,
                )
                # circular halos for the moving-side tiles
                nc.vector.tensor_copy(b_t[:, 0:HALO], b_t[:, J : J + HALO])
                nc.vector.tensor_copy(b_t[:, HALO + J :], b_t[:, HALO : 2 * HALO])
                nc.vector.tensor_copy(mb_t[:, 0:HALO], mb_t[:, J : J + HALO])
                nc.vector.tensor_copy(mb_t[:, HALO + J :], mb_t[:, HALO : 2 * HALO])

                for c in range(N_CHUNKS):
                    a0 = c * 128
                    first = g == 0 and c == 0
                    last = g == N_GROUPS - 1 and c == N_CHUNKS - 1
                    nc.tensor.matmul(
                        ps_ab[:],
                        a_t[:, a0 : a0 + 128].bitcast(f8),
                        b_t[:, a0 : a0 + NW].bitcast(f8),
                        start=first,
                        stop=last,
                    )
                    nc.tensor.matmul(
                        ps_mm[:],
                        ma_t[:, a0 : a0 + 128].bitcast(f8),
                        mb_t[:, a0 : a0 + NW].bitcast(f8),
                        start=first,
                        stop=last,
                    )

            out_sb = enc_pool.tile([128, 2, NW], f32, tag="out")
            nc.vector.tensor_copy(out_sb[:, 0], ps_ab[:])
            nc.vector.tensor_copy(out_sb[:, 1], ps_mm[:])
            nc.sync.dma_start(out_d[0], out_sb[:, 0])
            nc.sync.dma_start(out_d[1], out_sb[:, 1])

    import bass_rust as _bass_rust

    _bass_rust.move_matmul_waits_to_ldweights(nc.m)
    _bass_rust.generate_event_semaphores(nc)
    return nc


def _get_program():
    if "nc" not in _CACHE:
        _CACHE["nc"] = _build_program()
    return _CACHE["nc"]


def _shard(x):
    x = np.asarray(x)
    if x.dtype != np.uint8:
        x = x.view(np.uint8) if x.dtype == np.bool_ else x.astype(np.uint8)
    return [
        np.ascontiguousarray(x[:, c * B_SH : (c + 1) * B_SH]).reshape(ROWS, J)
        for c in range(N_CORES)
    ]


def kernel(iris_codes_a, mask_codes_a, iris_codes_b, mask_codes_b, _trace=False):
    from concourse.bass_utils import run_bass_kernel_spmd

    nc = _get_program()
    shards = {
        "ia": _shard(iris_codes_a),
        "ma": _shard(mask_codes_a),
        "ib": _shard(iris_codes_b),
        "mb": _shard(mask_codes_b),
    }
    in_maps = [{k: v[c] for k, v in shards.items()} for c in range(N_CORES)]
    res = run_bass_kernel_spmd(nc, in_maps, list(range(N_CORES)), trace=_trace)
    _CACHE["last_result"] = res

    acc = np.zeros((2, 128, NW), np.float64)
    for r in res.results:
        acc += r["out"].astype(np.float64)

    shifts = np.arange(-R, R + 1)
    cab = np.array([np.trace(acc[0], offset=HALO + 2 * s) for s in shifts])
    den = np.array([np.trace(acc[1], offset=HALO + 2 * s) for s in shifts])
    cab = np.rint(cab * 2.0**18)
    den = np.rint(den * 2.0**18)
    num = (den - cab) / 2.0
    dist = num.astype(np.float32) / den.astype(np.float32)
    out = np.minimum(np.float32(1.0), dist.min())
    return np.asarray([out], dtype=np.float32)



# revision 2
# speedup vs baseline: 1.3293x; 1.3293x over previous
"""Masked fractional Hamming distance over 31 circular rotations, on 8 trn2 cores.

Math: for shift s, num(s)/den(s) with
  den(s) = sum maskbits, num(s) = masked differing bits.
Encode A = (ia<<7)|ma, B = (ib<<7)|mb on the host; read as fp8e4m3 the bytes
become {+0, -0, +2^-9, -2^-9} (sign=iris, magnitude=mask), so
  corr(A, B)(s)      = (den - 2*num) * 2^-18
  corr(A&7F, B&7F)(s) = den * 2^-18        (mask strip done on-device)
Only A and B ship to the device (half the HBM traffic of raw ia/ma/ib/mb).

The fused (l,k) code axis only ever shifts by even amounts, so the host
deinterleaves it into two parity streams of length 2048; each stream needs
lags -15..+15 only. Correlations are banded matmuls on the PE: contraction
over rows (128/partition group), stationary = 128-wide halo'd window of the
B side (exactly 128 cols -> fast weight load), moving = 98-wide chunk of the
A side; every chunk/parity/row-group accumulates into one (128,98) PSUM tile
per pair since the diagonal offset d = w - i is tiling-invariant. Parity
streams are zero-padded 2048->2058 = 21*98 so all matmuls are uniform.
Band diagonals are summed on the host (exact integers scaled by 2^-18).
"""

import numpy as np

N_CORES = 8
B_FULL, L = 4096, 2048
R = 15
B_SH = B_FULL // N_CORES       # 512 batches per core
ROWS = 2 * B_SH                # 1024 rows per core
N_GROUPS = ROWS // 128         # 8
W = 98                         # moving chunk width
WIN = 128                      # stationary window width (exact 128 -> FWL)
HL, HR = 15, 25                # halos on each B parity segment
LP = 2058                      # padded parity stream length = 21 * W
N_CHUNKS = LP // W             # 21
SEG = HL + L + HR              # 2088
JA = 2 * LP                    # 4116 bytes per A row
JB = 2 * SEG                   # 4176 bytes per B row

_CACHE = {}


def _build_program():
    import concourse.bass as bass
    import concourse.tile as tile
    from concourse import mybir

    u8 = mybir.dt.uint8
    u16 = mybir.dt.uint16
    f8 = mybir.dt.float8e4
    f32 = mybir.dt.float32
    Alu = mybir.AluOpType

    nc = bass.Bass()
    a_d = nc.declare_dram_parameter("a", [ROWS, JA], u8, isOutput=False)
    b_d = nc.declare_dram_parameter("b", [ROWS, JB], u8, isOutput=False)
    out_d = nc.declare_dram_parameter("out", [2, WIN, W], f32, isOutput=True)

    with tile.TileContext(nc) as tc:
        with (
            tc.tile_pool(name="raw", bufs=3) as raw_pool,
            tc.tile_pool(name="msk", bufs=3) as msk_pool,
            tc.tile_pool(name="acc", bufs=1, space="PSUM") as psum_pool,
        ):
            ps_ab = psum_pool.tile([WIN, W], f32)
            ps_mm = psum_pool.tile([WIN, W], f32)

            for g in range(N_GROUPS):
                rows = slice(g * 128, (g + 1) * 128)
                a_t = raw_pool.tile([128, JA], u8, tag="a")
                b_t = raw_pool.tile([128, JB], u8, tag="b")
                ma_t = msk_pool.tile([128, JA], u8, tag="ma")
                mb_t = msk_pool.tile([128, JB], u8, tag="mb")

                nc.sync.dma_start(a_t[:], a_d[rows, :])
                nc.sync.dma_start(b_t[:], b_d[rows, :])

                # strip sign bit to recover masks: m = x & 0x7F (u16-packed)
                nc.vector.tensor_scalar(
                    ma_t[:].bitcast(u16), a_t[:].bitcast(u16),
                    0x7F7F, None, op0=Alu.bitwise_and,
                )
                nc.vector.tensor_scalar(
                    mb_t[:].bitcast(u16), b_t[:].bitcast(u16),
                    0x7F7F, None, op0=Alu.bitwise_and,
                )

                for k in range(2):
                    for c in range(N_CHUNKS):
                        p0 = c * W
                        oa = k * LP + p0
                        ob = k * SEG + p0
                        first = g == 0 and k == 0 and c == 0
                        last = g == N_GROUPS - 1 and k == 1 and c == N_CHUNKS - 1
                        nc.tensor.matmul(
                            ps_ab[:],
                            b_t[:, ob : ob + WIN].bitcast(f8),
                            a_t[:, oa : oa + W].bitcast(f8),
                            start=first,
                            stop=last,
                        )
                        nc.tensor.matmul(
                            ps_mm[:],
                            mb_t[:, ob : ob + WIN].bitcast(f8),
                            ma_t[:, oa : oa + W].bitcast(f8),
                            start=first,
                            stop=last,
                        )

            out_sb = msk_pool.tile([WIN, 2, W], f32, tag="out")
            nc.vector.tensor_copy(out_sb[:, 0], ps_ab[:])
            nc.vector.tensor_copy(out_sb[:, 1], ps_mm[:])
            nc.sync.dma_start(out_d[0], out_sb[:, 0])
            nc.sync.dma_start(out_d[1], out_sb[:, 1])

    import bass_rust as _bass_rust

    _bass_rust.move_matmul_waits_to_ldweights(nc.m)
    _bass_rust.generate_event_semaphores(nc)
    return nc


def _get_program():
    if "nc" not in _CACHE:
        _CACHE["nc"] = _build_program()
    return _CACHE["nc"]


def _as_u8(x):
    x = np.asarray(x)
    return x.view(np.uint8) if x.dtype == np.bool_ else x.astype(np.uint8)


def _prep(iris, mask, halo):
    """Encode (iris<<7)|mask, shard by batch, parity-deinterleave; per-core
    layout [ROWS, 2, L] -> zero-padded to LP (A side) or halo'd to SEG (B)."""
    enc = (_as_u8(iris) << 7) | _as_u8(mask)    # (2, B_FULL, L, 2)
    shards = []
    for c in range(N_CORES):
        s = enc[:, c * B_SH : (c + 1) * B_SH]   # (2, B_SH, L, 2)
        s = s.reshape(ROWS, L, 2).transpose(0, 2, 1)  # (ROWS, k, l)
        if halo:
            s = np.concatenate([s[:, :, -HL:], s, s[:, :, :HR]], axis=2)
        else:
            pad = np.zeros((ROWS, 2, LP - L), np.uint8)
            s = np.concatenate([s, pad], axis=2)
        shards.append(np.ascontiguousarray(s).reshape(ROWS, -1))
    return shards


def kernel(iris_codes_a, mask_codes_a, iris_codes_b, mask_codes_b, _trace=False):
    from concourse.bass_utils import run_bass_kernel_spmd

    nc = _get_program()
    a_sh = _prep(iris_codes_a, mask_codes_a, halo=False)
    b_sh = _prep(iris_codes_b, mask_codes_b, halo=True)
    in_maps = [{"a": a_sh[c], "b": b_sh[c]} for c in range(N_CORES)]
    res = run_bass_kernel_spmd(nc, in_maps, list(range(N_CORES)), trace=_trace)
    _CACHE["last_result"] = res

    acc = np.zeros((2, WIN, W), np.float64)
    for r in res.results:
        acc += r["out"].astype(np.float64)

    shifts = np.arange(-R, R + 1)
    ii = np.arange(W)
    cab = np.array([acc[0, ii + HL + s, ii].sum() for s in shifts])
    den = np.array([acc[1, ii + HL + s, ii].sum() for s in shifts])
    cab = np.rint(cab * 2.0**18)
    den = np.rint(den * 2.0**18)
    num = (den - cab) / 2.0
    dist = num.astype(np.float32) / den.astype(np.float32)
    out = np.minimum(np.float32(1.0), dist.min())
    return np.asarray([out], dtype=np.float32)


# revision 3
# speedup vs baseline: 1.4438x; 1.0861x over previous
"""Masked fractional Hamming distance over 31 circular rotations, on 8 trn2 cores.

Math: for shift s, num(s)/den(s) with
  den(s) = sum maskbits, num(s) = masked differing bits.
Encode A = (ia<<7)|ma, B = (ib<<7)|mb on the host; read as fp8e4m3 the bytes
become {+0, -0, +2^-9, -2^-9} (sign=iris, magnitude=mask), so
  corr(A, B)(s)       = (den - 2*num) * 2^-18
  corr(A&7F, B&7F)(s) = den * 2^-18       (mask strip done on-device)
Only A and B ship to the device (half the HBM traffic of raw ia/ma/ib/mb).

The fused (l,k) code axis only ever shifts by even amounts, so the host
deinterleaves it into two parity streams of length 2048; each stream needs
lags -15..+15 only. Correlations are banded matmuls on the PE: contraction
over rows (128/partition group), stationary = 128-wide halo'd window of the
B side (exactly 128 cols -> fast weight load), moving = 98-wide chunk of the
A side; every chunk/parity/row-group accumulates into one (128,98) PSUM tile
per pair since the diagonal offset d = w - i is tiling-invariant. Streams are
zero-padded so all 21 chunks are uniform and DMA rows stay 64B-aligned:
per-core row = [A0|B0|A1|B1], 4 x 2112 = 8448 bytes. Band diagonals are
summed on the host (exact integers scaled by 2^-18).
"""

import numpy as np

N_CORES = 8
B_FULL, L = 4096, 2048
R = 15
B_SH = B_FULL // N_CORES       # 512 batches per core
ROWS = 2 * B_SH                # 1024 rows per core
N_GROUPS = ROWS // 128         # 8
W = 98                         # moving chunk width
WIN = 128                      # stationary window width (exact 128 -> FWL)
HL, HR = 15, 25                # halos on the B parity segments
N_CHUNKS = 21                  # 21 * 98 = 2058 >= L
SEGP = 2112                    # padded segment bytes (mod 64 == 0)
HALF = 2 * SEGP                # 4224: one parity's [A|B] bytes
ROWB = 2 * HALF                # 8448 bytes per row

_CACHE = {}


def _build_program():
    import concourse.bass as bass
    import concourse.tile as tile
    from concourse import mybir

    u8 = mybir.dt.uint8
    u16 = mybir.dt.uint16
    f8 = mybir.dt.float8e4
    f32 = mybir.dt.float32
    Alu = mybir.AluOpType

    nc = bass.Bass()
    ab_d = nc.declare_dram_parameter("ab", [ROWS, ROWB], u8, isOutput=False)
    out_d = nc.declare_dram_parameter("out", [2, WIN, WIN], f32, isOutput=True)

    with tile.TileContext(nc) as tc:
        with (
            tc.tile_pool(name="raw", bufs=5) as raw_pool,
            tc.tile_pool(name="msk", bufs=3) as msk_pool,
            tc.tile_pool(name="acc", bufs=1, space="PSUM") as psum_pool,
        ):
            ps_ab = psum_pool.tile([WIN, W], f32)
            ps_mm = psum_pool.tile([WIN, W], f32)

            for g in range(N_GROUPS):
                rows = slice(g * 128, (g + 1) * 128)
                ab_t = raw_pool.tile([128, ROWB], u8, tag="ab")
                mab_t = msk_pool.tile([128, ROWB], u8, tag="mab")

                nc.sync.dma_start(ab_t[:, :HALF], ab_d[rows, :HALF])
                nc.scalar.dma_start(ab_t[:, HALF:], ab_d[rows, HALF:])

                # strip sign bit to recover masks: m = x & 0x7F (u16-packed)
                for k in range(2):
                    h = slice(k * HALF, (k + 1) * HALF)
                    nc.vector.tensor_scalar(
                        mab_t[:, h].bitcast(u16), ab_t[:, h].bitcast(u16),
                        0x7F7F, None, op0=Alu.bitwise_and,
                    )

                for k in range(2):
                    for c in range(N_CHUNKS):
                        p0 = c * W
                        oa = k * HALF + p0
                        ob = k * HALF + SEGP + p0
                        first = g == 0 and k == 0 and c == 0
                        last = g == N_GROUPS - 1 and k == 1 and c == N_CHUNKS - 1
                        nc.tensor.matmul(
                            ps_ab[:],
                            ab_t[:, ob : ob + WIN].bitcast(f8),
                            ab_t[:, oa : oa + W].bitcast(f8),
                            start=first,
                            stop=last,
                        )
                        nc.tensor.matmul(
                            ps_mm[:],
                            mab_t[:, ob : ob + WIN].bitcast(f8),
                            mab_t[:, oa : oa + W].bitcast(f8),
                            start=first,
                            stop=last,
                        )

            out_sb = msk_pool.tile([WIN, 2, WIN], f32, tag="out")
            nc.vector.tensor_copy(out_sb[:, 0, :W], ps_ab[:])
            nc.vector.tensor_copy(out_sb[:, 1, :W], ps_mm[:])
            nc.sync.dma_start(out_d[0], out_sb[:, 0])
            nc.scalar.dma_start(out_d[1], out_sb[:, 1])

    import bass_rust as _bass_rust

    _bass_rust.move_matmul_waits_to_ldweights(nc.m)
    _bass_rust.generate_event_semaphores(nc)
    return nc


def _get_program():
    if "nc" not in _CACHE:
        _CACHE["nc"] = _build_program()
    return _CACHE["nc"]


def _as_u8(x):
    x = np.asarray(x)
    return x.view(np.uint8) if x.dtype == np.bool_ else x.astype(np.uint8)


def _prep_core(ea, eb, c):
    """Per-core [ROWS, ROWB] u8: rows = (d0,batch) of the core's batch slice,
    row = [A0|B0|A1|B1] with A = parity stream (zero-padded), B = halo'd."""
    sa = ea[:, c * B_SH : (c + 1) * B_SH].reshape(ROWS, L, 2).transpose(0, 2, 1)
    sb = eb[:, c * B_SH : (c + 1) * B_SH].reshape(ROWS, L, 2).transpose(0, 2, 1)
    buf = np.zeros((ROWS, 2, 2, SEGP), np.uint8)
    buf[:, :, 0, :L] = sa
    buf[:, :, 1, :HL] = sb[:, :, -HL:]
    buf[:, :, 1, HL : HL + L] = sb
    buf[:, :, 1, HL + L : HL + L + HR] = sb[:, :, :HR]
    return buf.reshape(ROWS, ROWB)


def kernel(iris_codes_a, mask_codes_a, iris_codes_b, mask_codes_b, _trace=False):
    from concourse.bass_utils import run_bass_kernel_spmd

    nc = _get_program()
    ea = (_as_u8(iris_codes_a) << 7) | _as_u8(mask_codes_a)
    eb = (_as_u8(iris_codes_b) << 7) | _as_u8(mask_codes_b)
    in_maps = [{"ab": _prep_core(ea, eb, c)} for c in range(N_CORES)]
    res = run_bass_kernel_spmd(nc, in_maps, list(range(N_CORES)), trace=_trace)
    _CACHE["last_result"] = res

    acc = np.zeros((2, WIN, WIN), np.float64)
    for r in res.results:
        acc += r["out"].astype(np.float64)

    shifts = np.arange(-R, R + 1)
    ii = np.arange(W)
    cab = np.array([acc[0, ii + HL + s, ii].sum() for s in shifts])
    den = np.array([acc[1, ii + HL + s, ii].sum() for s in shifts])
    cab = np.rint(cab * 2.0**18)
    den = np.rint(den * 2.0**18)
    num = (den - cab) / 2.0
    dist = num.astype(np.float32) / den.astype(np.float32)
    out = np.minimum(np.float32(1.0), dist.min())
    return np.asarray([out], dtype=np.float32)
